# revision 6
# baseline (speedup 1.0000x reference)
"""Trainium2 Bass kernel for an AttentionBlock (GroupNorm + QKV + MHA + proj
+ residual), data-parallel over the batch across 8 NeuronCores.

Contract: kernel(**inputs) takes the FULL inputs of reference.setup_inputs()
and returns the FULL [8, 512, 32, 32] float32 output.

Per-core layout (core i handles batch element i, x viewed as [C=512, L=1024]):
  - GroupNorm(32 groups) via bn_stats per channel + tiny group-reduce matmuls.
  - QKV matmul in float32r with output rows permuted into pair-chunks
    [q0;q1]..[q6;q7], [k0;k1].., [v0;v1].. so that the two heads of a pair
    occupy partitions 0:64 / 64:128 of one 128-partition chunk.
  - scoresT[s, t] = k^T q per head, two heads packed in the PE array
    concurrently (K=64 row tiling via base partitions 0 / 64).
  - softmax without max-subtraction (logits are O(1) for this model):
    exp on the Scalar engine straight out of PSUM (fp32 -> bf16, FD=2048).
  - AV matmul with stationary operand [vT | ones] so rows 64:128 of the
    output accumulate sum(exp) per (head, t); normalization is a DVE divide.
  - proj matmul in float32r + fused (bias + residual) via scalar_tensor_tensor.
"""

import contextlib

import numpy as np

import concourse.bass as bass
import concourse.tile as tile
from concourse import mybir
from concourse.bass_utils import run_bass_kernel_spmd

F32 = mybir.dt.float32
F32R = mybir.dt.float32r
BF16 = mybir.dt.bfloat16
FT = mybir.ActivationFunctionType
ALU = mybir.AluOpType

B, C, HH, WW = 8, 512, 32, 32
L = HH * WW            # 1024
NH = 8                 # heads
CH = C // NH           # 64 channels per head
NG = 32                # groupnorm groups
GS = C // NG           # 16 channels per group
EPS = 1e-5
NCHUNK = C // 128      # 4 partition chunks of channels
N_CORES = 8


def _split_excess_waits(nc, default_max=1, ctrl_max=1):
    """walrus only encodes 1 sync wait on CTRL-like instructions (Drain/NoOp)
    and 2 on regular ones; split extra waits onto preceding NoOp carriers."""
    n_split = 0
    for f in nc.m.functions:
        for bb in f.blocks:
            insts = bb.instructions
            i = 0
            while i < len(insts):
                inst = insts[i]
                si = inst.sync_info
                cap = (
                    ctrl_max
                    if isinstance(inst, (mybir.InstDrain, mybir.InstNoOp))
                    else default_max
                )
                if si is not None and si.on_wait and len(si.on_wait) > cap:
                    waits = list(si.on_wait)
                    keep, extra = waits[-cap:], waits[:-cap]
                    carriers = [
                        mybir.InstNoOp(
                            name=f"{inst.name}-wsplit-{j}",
                            engine=inst.engine,
                            sync_info=mybir.SyncInfo(
                                on_wait=[w], on_update=[]
                            ),
                            bass_nofuse=True,
                        )
                        for j, w in enumerate(extra)
                    ]
                    inst.sync_info = mybir.SyncInfo(
                        on_wait=keep, on_update=list(si.on_update or [])
                    )
                    for k, c in enumerate(carriers):
                        insts.insert(i + k, c)
                    i += len(carriers)
                    n_split += 1
                i += 1
    return n_split


def build_nc(split_waits=True):
    nc = bass.Bass("TRN2", debug=False)

    x_d = nc.dram_tensor("x", [C, L], F32, kind="ExternalInput")
    qkvt_d = nc.dram_tensor("qkvt", [C, 3 * C], F32R, kind="ExternalInput")
    qkb_d = nc.dram_tensor("qkb", [128, 8], F32, kind="ExternalInput")
    projt_d = nc.dram_tensor("projt", [C, C], F32R, kind="ExternalInput")
    projb_d = nc.dram_tensor("projb", [128, NCHUNK], F32, kind="ExternalInput")
    gnw_d = nc.dram_tensor("gnw", [128, NCHUNK], F32, kind="ExternalInput")
    gnb_d = nc.dram_tensor("gnb", [128, NCHUNK], F32, kind="ExternalInput")
    gnind_d = nc.dram_tensor("gnind", [128, NCHUNK * NG], F32, kind="ExternalInput")
    gnexp_d = nc.dram_tensor("gnexp", [NG, NCHUNK * 128], F32, kind="ExternalInput")
    out_d = nc.dram_tensor("out", [C, L], F32, kind="ExternalOutput")

    with tile.TileContext(nc) as tc, contextlib.ExitStack() as top:
        consts = top.enter_context(tc.tile_pool(name="consts", bufs=1))
        xpool = top.enter_context(tc.tile_pool(name="x", bufs=1))
        qkpool = top.enter_context(tc.tile_pool(name="qk", bufs=1))
        vtpool = top.enter_context(tc.tile_pool(name="vt", bufs=1))
        apool = top.enter_context(tc.tile_pool(name="a", bufs=1))

        # ---- constant loads -------------------------------------------------
        wq = []
        for c in range(NCHUNK):
            t = consts.tile([128, 3 * C], F32R, tag=f"wq{c}")
            nc.sync.dma_start(out=t, in_=qkvt_d.ap()[c * 128:(c + 1) * 128, :])
            wq.append(t)
        pw = []
        for c in range(NCHUNK):
            t = consts.tile([128, C], F32R, tag=f"pw{c}")
            nc.sync.dma_start(out=t, in_=projt_d.ap()[c * 128:(c + 1) * 128, :])
            pw.append(t)
        qkb = consts.tile([128, 8], F32)
        nc.sync.dma_start(out=qkb, in_=qkb_d.ap())
        projb = consts.tile([128, NCHUNK], F32)
        nc.sync.dma_start(out=projb, in_=projb_d.ap())
        gnw = consts.tile([128, NCHUNK], F32)
        nc.sync.dma_start(out=gnw, in_=gnw_d.ap())
        gnb = consts.tile([128, NCHUNK], F32)
        nc.sync.dma_start(out=gnb, in_=gnb_d.ap())
        gnind = consts.tile([128, NCHUNK * NG], F32)
        nc.sync.dma_start(out=gnind, in_=gnind_d.ap())
        gnexp = consts.tile([NG, NCHUNK * 128], F32)
        nc.sync.dma_start(out=gnexp, in_=gnexp_d.ap())
        epsv = consts.tile([NG, 1], F32)
        nc.vector.memset(epsv, EPS)

        xs = []
        for c in range(NCHUNK):
            t = xpool.tile([128, L], F32, tag=f"x{c}")
            nc.sync.dma_start(out=t, in_=x_d.ap()[c * 128:(c + 1) * 128, :])
            xs.append(t)

        # ---- GroupNorm ------------------------------------------------------
        with contextlib.ExitStack() as gctx:
            gsb = gctx.enter_context(tc.tile_pool(name="gn_sb", bufs=1))
            gps = gctx.enter_context(tc.tile_pool(name="gn_ps", bufs=2, space="PSUM"))

            stats3 = gsb.tile([128, NCHUNK, 3], F32)
            for c in range(NCHUNK):
                st6 = gsb.tile([128, 2, 6], F32, tag="st6")
                nc.vector.bn_stats(out=st6[:, 0, :], in_=xs[c][:, 0:512])
                nc.vector.bn_stats(out=st6[:, 1, :], in_=xs[c][:, 512:1024])
                mv = gsb.tile([128, 2], F32, tag="mv")
                nc.vector.bn_aggr(out=mv, in_=st6)
                nc.vector.tensor_copy(stats3[:, c, 0:2], mv)
                nc.vector.tensor_tensor(
                    out=stats3[:, c, 2:3], in0=mv[:, 0:1], in1=mv[:, 0:1],
                    op=ALU.mult,
                )
            gst = gps.tile([NG, 3], F32)
            for c in range(NCHUNK):
                nc.tensor.matmul(
                    gst,
                    lhsT=gnind[:, c * NG:(c + 1) * NG],
                    rhs=stats3[:, c, :],
                    start=(c == 0), stop=(c == NCHUNK - 1),
                )
            # group stats: [gmean, mean_of_var, mean_of_mean2]
            grs = gsb.tile([NG, 3], F32)
            nc.vector.tensor_copy(grs, gst)
            gvar = gsb.tile([NG, 1], F32)
            nc.vector.tensor_tensor(out=gvar, in0=grs[:, 1:2], in1=grs[:, 2:3], op=ALU.add)
            m2 = gsb.tile([NG, 1], F32)
            nc.vector.tensor_tensor(out=m2, in0=grs[:, 0:1], in1=grs[:, 0:1], op=ALU.mult)
            nc.vector.tensor_tensor(out=gvar, in0=gvar, in1=m2, op=ALU.subtract)
            # grs2: col0 = gmean, col1 = rstd
            grs2 = gsb.tile([NG, 2], F32)
            nc.vector.tensor_copy(grs2[:, 0:1], grs[:, 0:1])
            sd = gsb.tile([NG, 1], F32)
            nc.scalar.activation(out=sd, in_=gvar, func=FT.Sqrt, bias=epsv, scale=1.0)
            nc.vector.reciprocal(out=grs2[:, 1:2], in_=sd)

            alpha = gsb.tile([128, NCHUNK], F32)
            beta = gsb.tile([128, NCHUNK], F32)
            for c in range(NCHUNK):
                cs = gps.tile([128, 2], F32, tag="cs")
                nc.tensor.matmul(
                    cs,
                    lhsT=gnexp[:, c * 128:(c + 1) * 128],
                    rhs=grs2,
                    start=True, stop=True,
                )
                nc.vector.tensor_tensor(
                    out=alpha[:, c:c + 1], in0=cs[:, 1:2], in1=gnw[:, c:c + 1],
                    op=ALU.mult,
                )
                ngm = gsb.tile([128, 1], F32, tag="ngm")
                nc.vector.tensor_scalar(
                    out=ngm, in0=cs[:, 0:1], scalar1=-1.0, scalar2=None, op0=ALU.mult,
                )
                nc.vector.scalar_tensor_tensor(
                    out=beta[:, c:c + 1], in0=ngm, scalar=alpha[:, c:c + 1],
                    in1=gnb[:, c:c + 1], op0=ALU.mult, op1=ALU.add,
                )
            # xn = x * alpha + beta  (float32, bitcast to f32r at matmul time)
            xn = []
            for c in range(NCHUNK):
                t = qkpool.tile([128, L], F32R, tag=f"xn{c}")
                nc.vector.tensor_scalar(
                    out=t, in0=xs[c], scalar1=alpha[:, c:c + 1],
                    scalar2=beta[:, c:c + 1], op0=ALU.mult, op1=ALU.add,
                )
                xn.append(t)

        # ---- QKV matmul (f32r), drain q/k with bias->bf16, v plain->bf16 ----
        qp, kp, vp = [], [], []
        with contextlib.ExitStack() as qctx:
            qps = qctx.enter_context(tc.tile_pool(name="qkv_ps", bufs=3, space="PSUM"))
            for m in range(12):
                pt = qps.tile([128, L], F32, tag="qkvps")
                for n in range(2):
                    for c in range(NCHUNK):
                        nc.tensor.matmul(
                            pt[:, n * 512:(n + 1) * 512],
                            lhsT=wq[c][:, m * 128:(m + 1) * 128],
                            rhs=xn[c][:, n * 512:(n + 1) * 512],
                            start=(c == 0), stop=(c == NCHUNK - 1),
                        )
                if m < 8:
                    t = qkpool.tile([128, L], BF16, tag=f"qk{m}")
                    nc.vector.tensor_scalar(
                        out=t, in0=pt, scalar1=qkb[:, m:m + 1], scalar2=None,
                        op0=ALU.add,
                    )
                    (qp if m < 4 else kp).append(t)
                else:
                    t = qkpool.tile([128, L], BF16, tag=f"v{m}")
                    nc.scalar.copy(out=t, in_=pt)
                    vp.append(t)

        # ---- vT' tiles: [s, 0:64]=v_h^T, [s, 64:128]=ones -------------------
        vth = []
        for h in range(NH):
            t = vtpool.tile([128, 8, 128], BF16, tag=f"vt{h}")
            nc.gpsimd.memset(t[:, :, 64:128], 1.0)
            vth.append(t)
        for j in range(NCHUNK):
            for i in range(8):
                nc.sync.dma_start(
                    out=vth[2 * j][:, i, 0:64],
                    in_=vp[j][0:64, i * 128:(i + 1) * 128], transpose=True,
                )
                nc.sync.dma_start(
                    out=vth[2 * j + 1][:, i, 0:64],
                    in_=vp[j][64:128, i * 128:(i + 1) * 128], transpose=True,
                )

        # ---- attention per head-pair ---------------------------------------
        ach = []
        with contextlib.ExitStack() as actx:
            sps = actx.enter_context(tc.tile_pool(name="sc_ps", bufs=2, space="PSUM"))
            wtp_pool = actx.enter_context(tc.tile_pool(name="wt", bufs=1))
            sespool = actx.enter_context(tc.tile_pool(name="ses", bufs=2))
            for p in range(NCHUNK):
                wt = wtp_pool.tile([128, 8, 2048], BF16, tag="wt")
                for i in range(8):
                    st = sps.tile([128, 2048], F32, tag="scav")
                    for hb, off in ((0, 0), (64, 1024)):
                        for n in range(2):
                            nc.tensor.matmul(
                                st[:, off + n * 512: off + (n + 1) * 512],
                                lhsT=kp[p][hb:hb + 64, i * 128:(i + 1) * 128],
                                rhs=qp[p][hb:hb + 64, n * 512:(n + 1) * 512],
                                start=True, stop=True,
                            )
                    nc.scalar.activation(out=wt[:, i, :], in_=st, func=FT.Exp)
                av = sps.tile([128, 2048], F32, tag="scav")
                for i in range(8):
                    for hi, off in ((2 * p, 0), (2 * p + 1, 1024)):
                        for n in range(2):
                            nc.tensor.matmul(
                                av[:, off + n * 512: off + (n + 1) * 512],
                                lhsT=vth[hi][:, i, :],
                                rhs=wt[:, i, off + n * 512: off + (n + 1) * 512],
                                start=(i == 0), stop=(i == 7),
                            )
                ses = sespool.tile([64, 2048], F32, tag="ses")
                nc.vector.reciprocal(out=ses, in_=av[64:128, :])
                a_t = apool.tile([128, L], F32R, tag=f"a{p}")
                nc.vector.tensor_tensor(
                    out=a_t[0:64, :], in0=av[0:64, 0:1024], in1=ses[0:64, 0:1024],
                    op=ALU.mult,
                )
                nc.vector.tensor_tensor(
                    out=a_t[64:128, :], in0=av[0:64, 1024:2048],
                    in1=ses[0:64, 1024:2048], op=ALU.mult,
                )
                ach.append(a_t)

        # ---- proj + bias + residual ----------------------------------------
        with contextlib.ExitStack() as pctx:
            pps = pctx.enter_context(tc.tile_pool(name="pr_ps", bufs=2, space="PSUM"))
            opool = pctx.enter_context(tc.tile_pool(name="o", bufs=2))
            for m in range(NCHUNK):
                pt = pps.tile([128, L], F32, tag="prps")
                for n in range(2):
                    for c in range(NCHUNK):
                        nc.tensor.matmul(
                            pt[:, n * 512:(n + 1) * 512],
                            lhsT=pw[c][:, m * 128:(m + 1) * 128],
                            rhs=ach[c][:, n * 512:(n + 1) * 512],
                            start=(c == 0), stop=(c == NCHUNK - 1),
                        )
                ot = opool.tile([128, L], F32, tag="ot")
                nc.vector.scalar_tensor_tensor(
                    out=ot, in0=pt, scalar=projb[:, m:m + 1], in1=xs[m],
                    op0=ALU.add, op1=ALU.add,
                )
                nc.sync.dma_start(
                    out=out_d.ap()[m * 128:(m + 1) * 128, :], in_=ot,
                )

    if split_waits:
        _split_excess_waits(nc)
    return nc


def prep_inputs(x, gn_w, gn_b, qkv_w, qkv_b, proj_w, proj_b):
    """Host-side prep: permute/scale QKV weights, fold biases, GN indicators."""
    x = np.ascontiguousarray(np.asarray(x, dtype=np.float32)).reshape(B, C, L)
    qkv_w = np.asarray(qkv_w, dtype=np.float32)
    qkv_b = np.asarray(qkv_b, dtype=np.float32)
    proj_w = np.asarray(proj_w, dtype=np.float32)
    proj_b = np.asarray(proj_b, dtype=np.float32)
    gn_w = np.asarray(gn_w, dtype=np.float32)
    gn_b = np.asarray(gn_b, dtype=np.float32)

    # output-row permutation: q pair-chunks, k pair-chunks, v pair-chunks
    perm = np.empty(3 * C, dtype=np.int64)
    pos = 0
    for part in range(3):             # 0=q, 1=k, 2=v
        for h in range(NH):
            rows = h * 3 * CH + part * CH + np.arange(CH)
            perm[pos:pos + CH] = rows
            pos += CH
    w_perm = qkv_w[perm, :].copy()
    b_perm = qkv_b[perm].copy()
    w_perm[0:C] *= 0.125              # fold softmax scale^2 into q
    b_perm[0:C] *= 0.125

    qkvt = np.ascontiguousarray(w_perm.T)                      # [C, 3C]
    qkb = np.ascontiguousarray(b_perm[0:2 * C].reshape(8, 128).T)  # [128, 8]
    bv = b_perm[2 * C:3 * C]                                   # v bias, head-major == channel order
    projt = np.ascontiguousarray(proj_w.T)                     # [C, C]
    projb = np.ascontiguousarray(
        (proj_b + proj_w @ bv).reshape(NCHUNK, 128).T)         # [128, 4]
    gnw_t = np.ascontiguousarray(gn_w.reshape(NCHUNK, 128).T)  # [128, 4]
    gnb_t = np.ascontiguousarray(gn_b.reshape(NCHUNK, 128).T)

    gnind = np.zeros((128, NCHUNK * NG), np.float32)
    gnexp = np.zeros((NG, NCHUNK * 128), np.float32)
    for c in range(NCHUNK):
        for p in range(128):
            g = (c * 128 + p) // GS
            gnind[p, c * NG + g] = 1.0 / GS
            gnexp[g, c * 128 + p] = 1.0
    shared = {
        "qkvt": qkvt, "qkb": qkb, "projt": projt, "projb": projb,
        "gnw": gnw_t, "gnb": gnb_t, "gnind": gnind, "gnexp": gnexp,
    }
    in_maps = [
        {"x": np.ascontiguousarray(x[i]), **shared} for i in range(N_CORES)
    ]
    return in_maps


_NC_CACHE = {}


def _get_nc():
    if "nc" not in _NC_CACHE:
        _NC_CACHE["nc"] = build_nc()
    return _NC_CACHE["nc"]


def kernel(x, gn_w, gn_b, qkv_w, qkv_b, proj_w, proj_b, _trace=False, _tmpdir=None):
    nc = _get_nc()
    in_maps = prep_inputs(x, gn_w, gn_b, qkv_w, qkv_b, proj_w, proj_b)
    res = run_bass_kernel_spmd(
        nc, in_maps, core_ids=list(range(N_CORES)), trace=_trace, tmpdir=_tmpdir,
    )
    out = np.stack([res.results[i]["out"] for i in range(N_CORES)], axis=0)
    out = out.reshape(B, C, HH, WW).astype(np.float32)
    if _trace:
        kernel.last_results = res
    return out


# revision 16
# speedup vs baseline: 1.2152x; 1.2152x over previous
"""Trainium2 Bass kernel for an AttentionBlock (GroupNorm + QKV + MHA + proj
+ residual), data-parallel over the batch across 8 NeuronCores.

Contract: kernel(**inputs) takes the FULL inputs of reference.setup_inputs()
and returns the FULL [8, 512, 32, 32] float32 output.

Per-core layout (core i handles batch element i, x viewed as [C=512, L=1024]):
  - GroupNorm(32 groups) via bn_stats per channel + tiny group-reduce matmuls.
  - QKV matmul in float32r with output rows permuted into pair-chunks
    [q0;q1]..[q6;q7], [k0;k1].., [v0;v1].. so that the two heads of a pair
    occupy partitions 0:64 / 64:128 of one 128-partition chunk.
  - scoresT[s, t] = k^T q per head, two heads packed in the PE array
    concurrently (K=64 row tiling via base partitions 0 / 64).
  - softmax without max-subtraction (logits are O(1) for this model):
    exp on the Scalar engine straight out of PSUM (fp32 -> bf16, FD=2048).
  - AV matmul with stationary operand [vT | ones] so rows 64:128 of the
    output accumulate sum(exp) per (head, t); normalization is a DVE divide.
  - proj matmul in float32r + fused (bias + residual) via scalar_tensor_tensor.
"""

import contextlib

import numpy as np

import concourse.bass as bass
import concourse.tile as tile
from concourse import mybir
from concourse.bass_utils import run_bass_kernel_spmd

F32 = mybir.dt.float32
F32R = mybir.dt.float32r
BF16 = mybir.dt.bfloat16
FT = mybir.ActivationFunctionType
ALU = mybir.AluOpType

B, C, HH, WW = 8, 512, 32, 32
L = HH * WW            # 1024
NH = 8                 # heads
CH = C // NH           # 64 channels per head
NG = 32                # groupnorm groups
GS = C // NG           # 16 channels per group
EPS = 1e-5
NCHUNK = C // 128      # 4 partition chunks of channels
N_CORES = 8


def _split_excess_waits(nc, default_max=1, ctrl_max=1):
    """walrus only encodes 1 sync wait on CTRL-like instructions (Drain/NoOp)
    and 2 on regular ones; split extra waits onto preceding NoOp carriers."""
    n_split = 0
    for f in nc.m.functions:
        for bb in f.blocks:
            insts = bb.instructions
            i = 0
            while i < len(insts):
                inst = insts[i]
                si = inst.sync_info
                cap = (
                    ctrl_max
                    if isinstance(inst, (mybir.InstDrain, mybir.InstNoOp))
                    else default_max
                )
                if si is not None and si.on_wait and len(si.on_wait) > cap:
                    waits = list(si.on_wait)
                    keep, extra = waits[-cap:], waits[:-cap]
                    carriers = [
                        mybir.InstNoOp(
                            name=f"{inst.name}-wsplit-{j}",
                            engine=inst.engine,
                            sync_info=mybir.SyncInfo(
                                on_wait=[w], on_update=[]
                            ),
                            bass_nofuse=True,
                        )
                        for j, w in enumerate(extra)
                    ]
                    inst.sync_info = mybir.SyncInfo(
                        on_wait=keep, on_update=list(si.on_update or [])
                    )
                    for k, c in enumerate(carriers):
                        insts.insert(i + k, c)
                    i += len(carriers)
                    n_split += 1
                i += 1
    return n_split


def build_nc(split_waits=True):
    nc = bass.Bass("TRN2", debug=False)

    x_d = nc.dram_tensor("x", [C, L], F32, kind="ExternalInput")
    qkvt_d = nc.dram_tensor("qkvt", [C, 3 * C], F32R, kind="ExternalInput")
    qkb_d = nc.dram_tensor("qkb", [128, 8], F32, kind="ExternalInput")
    projt_d = nc.dram_tensor("projt", [C, C], F32R, kind="ExternalInput")
    projb_d = nc.dram_tensor("projb", [128, NCHUNK], F32, kind="ExternalInput")
    gnw_d = nc.dram_tensor("gnw", [128, NCHUNK], F32, kind="ExternalInput")
    gnb_d = nc.dram_tensor("gnb", [128, NCHUNK], F32, kind="ExternalInput")
    gnind_d = nc.dram_tensor("gnind", [128, NCHUNK * NG], F32, kind="ExternalInput")
    gnexp_d = nc.dram_tensor("gnexp", [NG, NCHUNK * 128], F32, kind="ExternalInput")
    out_d = nc.dram_tensor("out", [C, L], F32, kind="ExternalOutput")
    ses_d = nc.dram_tensor("sesdram", [NCHUNK, 2, 2048], F32)

    with tile.TileContext(nc) as tc, contextlib.ExitStack() as top:
        consts = top.enter_context(tc.tile_pool(name="consts", bufs=1))
        xpool = top.enter_context(tc.tile_pool(name="x", bufs=1))
        qkpool = top.enter_context(tc.tile_pool(name="qk", bufs=1))
        vtpool = top.enter_context(tc.tile_pool(name="vt", bufs=1))
        apool = top.enter_context(tc.tile_pool(name="a", bufs=1))
        qkv_stack = contextlib.ExitStack()
        wqpool = qkv_stack.enter_context(tc.tile_pool(name="wq", bufs=1))

        # ---- input loads (x first: GroupNorm is the critical path) ----------
        xs = []
        for c in range(NCHUNK):
            t = xpool.tile([128, L], F32, tag=f"x{c}")
            nc.sync.dma_start(out=t, in_=x_d.ap()[c * 128:(c + 1) * 128, :])
            xs.append(t)
        gnw = consts.tile([128, NCHUNK], F32)
        nc.sync.dma_start(out=gnw, in_=gnw_d.ap())
        gnb = consts.tile([128, NCHUNK], F32)
        nc.sync.dma_start(out=gnb, in_=gnb_d.ap())
        gnind = consts.tile([128, NCHUNK * NG], F32)
        nc.sync.dma_start(out=gnind, in_=gnind_d.ap())
        gnexp = consts.tile([NG, NCHUNK * 128], F32)
        nc.sync.dma_start(out=gnexp, in_=gnexp_d.ap())
        epsv = consts.tile([NG, 1], F32)
        nc.vector.memset(epsv, EPS)
        # prefetch the Sqrt activation table while DMAs run
        sqrt_warm = consts.tile([NG, 1], F32)
        nc.scalar.activation(out=sqrt_warm, in_=epsv, func=FT.Sqrt)

        wq = []
        for c in range(NCHUNK):
            t = wqpool.tile([128, 3 * C], F32R, tag=f"wq{c}")
            nc.sync.dma_start(out=t, in_=qkvt_d.ap()[c * 128:(c + 1) * 128, :])
            wq.append(t)
        pw = []
        for c in range(NCHUNK):
            t = consts.tile([128, C], F32R, tag=f"pw{c}")
            nc.sync.dma_start(out=t, in_=projt_d.ap()[c * 128:(c + 1) * 128, :])
            pw.append(t)
        qkb = consts.tile([128, 8], F32)
        nc.sync.dma_start(out=qkb, in_=qkb_d.ap())
        projb = consts.tile([128, NCHUNK], F32)
        nc.sync.dma_start(out=projb, in_=projb_d.ap())

        # ---- GroupNorm ------------------------------------------------------
        with contextlib.ExitStack() as gctx:
            gsb = gctx.enter_context(tc.tile_pool(name="gn_sb", bufs=1))
            gps = gctx.enter_context(tc.tile_pool(name="gn_ps", bufs=2, space="PSUM"))

            stats3 = gsb.tile([128, NCHUNK, 3], F32)
            for c in range(NCHUNK):
                st6 = gsb.tile([128, 2, 6], F32, tag="st6")
                nc.vector.bn_stats(out=st6[:, 0, :], in_=xs[c][:, 0:512])
                nc.vector.bn_stats(out=st6[:, 1, :], in_=xs[c][:, 512:1024])
                mv = gsb.tile([128, 2], F32, tag="mv")
                nc.vector.bn_aggr(out=mv, in_=st6)
                nc.vector.tensor_copy(stats3[:, c, 0:2], mv)
                nc.vector.tensor_tensor(
                    out=stats3[:, c, 2:3], in0=mv[:, 0:1], in1=mv[:, 0:1],
                    op=ALU.mult,
                )
            gst = gps.tile([NG, 3], F32)
            for c in range(NCHUNK):
                nc.tensor.matmul(
                    gst,
                    lhsT=gnind[:, c * NG:(c + 1) * NG],
                    rhs=stats3[:, c, :],
                    start=(c == 0), stop=(c == NCHUNK - 1),
                )
            # group stats: [gmean, mean_of_var, mean_of_mean2]
            grs = gsb.tile([NG, 3], F32)
            nc.vector.tensor_copy(grs, gst)
            gvar = gsb.tile([NG, 1], F32)
            nc.vector.tensor_tensor(out=gvar, in0=grs[:, 1:2], in1=grs[:, 2:3], op=ALU.add)
            m2 = gsb.tile([NG, 1], F32)
            nc.vector.tensor_tensor(out=m2, in0=grs[:, 0:1], in1=grs[:, 0:1], op=ALU.mult)
            nc.vector.tensor_tensor(out=gvar, in0=gvar, in1=m2, op=ALU.subtract)
            # grs2: col0 = gmean, col1 = rstd
            grs2 = gsb.tile([NG, 2], F32)
            nc.vector.tensor_copy(grs2[:, 0:1], grs[:, 0:1])
            sd = gsb.tile([NG, 1], F32)
            nc.scalar.activation(out=sd, in_=gvar, func=FT.Sqrt, bias=epsv, scale=1.0)
            nc.vector.reciprocal(out=grs2[:, 1:2], in_=sd)

            alpha = gsb.tile([128, NCHUNK], F32)
            beta = gsb.tile([128, NCHUNK], F32)
            for c in range(NCHUNK):
                cs = gps.tile([128, 2], F32, tag="cs")
                nc.tensor.matmul(
                    cs,
                    lhsT=gnexp[:, c * 128:(c + 1) * 128],
                    rhs=grs2,
                    start=True, stop=True,
                )
                nc.vector.tensor_tensor(
                    out=alpha[:, c:c + 1], in0=cs[:, 1:2], in1=gnw[:, c:c + 1],
                    op=ALU.mult,
                )
                ngm = gsb.tile([128, 1], F32, tag="ngm")
                nc.vector.tensor_scalar(
                    out=ngm, in0=cs[:, 0:1], scalar1=-1.0, scalar2=None, op0=ALU.mult,
                )
                nc.vector.scalar_tensor_tensor(
                    out=beta[:, c:c + 1], in0=ngm, scalar=alpha[:, c:c + 1],
                    in1=gnb[:, c:c + 1], op0=ALU.mult, op1=ALU.add,
                )
            # xn = x * alpha + beta  (float32r so it can feed f32r matmuls)
            xn = []
            for c in range(NCHUNK):
                t = wqpool.tile([128, L], F32R, tag=f"xn{c}")
                nc.vector.tensor_scalar(
                    out=t, in0=xs[c], scalar1=alpha[:, c:c + 1],
                    scalar2=beta[:, c:c + 1], op0=ALU.mult, op1=ALU.add,
                )
                xn.append(t)
            # prefetch the Exp activation table (overlaps the QKV matmuls)
            exp_warm = gsb.tile([NG, 1], F32)
            nc.scalar.activation(out=exp_warm, in_=sd, func=FT.Exp)

        # ---- QKV (f32r) interleaved per head-pair ---------------------------
        # q/k: standard [o-chunk, l] outputs, drained with bias -> bf16.
        # v: computed TRANSPOSED (xn stationary, v-weight cols moving) so the
        #    AV stationary operand [vT | ones] needs no separate transpose.
        qp, kp = [None] * 4, [None] * 4
        vth = []
        for h in range(NH):
            t = vtpool.tile([128, 8, 128], BF16, tag=f"vt{h}")
            nc.gpsimd.memset(t[:, :, 64:128], 1.0)
            vth.append(t)
        qps = qkv_stack.enter_context(tc.tile_pool(name="qkv_ps", bufs=3, space="PSUM"))
        vps = qkv_stack.enter_context(tc.tile_pool(name="vt_ps", bufs=2, space="PSUM"))
        for p in range(NCHUNK):
            for m in (p, 4 + p):
                pt = qps.tile([128, L], F32, tag="qkvps")
                for n in range(2):
                    for c in range(NCHUNK):
                        nc.tensor.matmul(
                            pt[:, n * 512:(n + 1) * 512],
                            lhsT=wq[c][:, m * 128:(m + 1) * 128],
                            rhs=xn[c][:, n * 512:(n + 1) * 512],
                            start=(c == 0), stop=(c == NCHUNK - 1),
                        )
                t = qkpool.tile([128, L], BF16, tag=f"qk{m}")
                nc.vector.tensor_scalar(
                    out=t, in0=pt, scalar1=qkb[:, m:m + 1], scalar2=None,
                    op0=ALU.add,
                )
                (qp if m < 4 else kp)[m % 4] = t
            # vT for this pair: out block [s-block, c-pair] per 4-block group
            vcols = slice(1024 + p * 128, 1024 + (p + 1) * 128)
            for g in range(2):
                vt_ps = vps.tile([128, 4, 128], F32, tag="vtps")
                for b in range(4):
                    i = g * 4 + b
                    for c in range(NCHUNK):
                        nc.tensor.matmul(
                            vt_ps[:, b, :],
                            lhsT=xn[c][:, i * 128:(i + 1) * 128],
                            rhs=wq[c][:, vcols],
                            start=(c == 0), stop=(c == NCHUNK - 1),
                        )
                nc.vector.tensor_copy(
                    vth[2 * p][:, g * 4:(g + 1) * 4, 0:64], vt_ps[:, :, 0:64],
                )
                nc.vector.tensor_copy(
                    vth[2 * p + 1][:, g * 4:(g + 1) * 4, 0:64], vt_ps[:, :, 64:128],
                )
        qkv_stack.close()

        # ---- attention per head-pair ---------------------------------------
        ach = []
        with contextlib.ExitStack() as actx:
            sps = actx.enter_context(tc.tile_pool(name="sc_ps", bufs=2, space="PSUM"))
            wtp_pool = actx.enter_context(tc.tile_pool(name="wt", bufs=2))
            aupool = actx.enter_context(tc.tile_pool(name="aun", bufs=2))
            rpool = actx.enter_context(tc.tile_pool(name="rcp", bufs=2))
            for p in range(NCHUNK):
                wt = wtp_pool.tile([128, 8, 2048], BF16, tag="wt")
                for i in range(8):
                    st = sps.tile([128, 2048], F32, tag="scav")
                    for hb, off in ((0, 0), (64, 1024)):
                        for n in range(2):
                            nc.tensor.matmul(
                                st[:, off + n * 512: off + (n + 1) * 512],
                                lhsT=kp[p][hb:hb + 64, i * 128:(i + 1) * 128],
                                rhs=qp[p][hb:hb + 64, n * 512:(n + 1) * 512],
                                start=True, stop=True,
                            )
                    nc.scalar.activation(out=wt[:, i, :], in_=st, func=FT.Exp)
                av = sps.tile([128, 2048], F32, tag="scav")
                for i in range(8):
                    for hi, off in ((2 * p, 0), (2 * p + 1, 1024)):
                        for n in range(2):
                            nc.tensor.matmul(
                                av[:, off + n * 512: off + (n + 1) * 512],
                                lhsT=vth[hi][:, i, :],
                                rhs=wt[:, i, off + n * 512: off + (n + 1) * 512],
                                start=(i == 0), stop=(i == 7),
                            )
                # [a_un; sumexp] out of PSUM in one copy, then spread-reciprocal
                aun = aupool.tile([65, 2048], F32, tag="aun")
                nc.vector.tensor_copy(aun, av[0:65, :])
                nc.sync.dma_start(out=ses_d.ap()[p, 0, :], in_=aun[64:65, :])
                sesw = rpool.tile([128, 16], F32, tag="sesw")
                nc.sync.dma_start(
                    out=sesw,
                    in_=ses_d.ap()[p, 0, :].rearrange("(p f) -> p f", p=128),
                )
                nc.vector.reciprocal(out=sesw, in_=sesw)
                nc.sync.dma_start(out=ses_d.ap()[p, 1, :], in_=sesw)
                rb = rpool.tile([64, 2048], F32, tag="rb")
                row = ses_d.ap()[p, 1, :]
                rb_src = bass.AP(
                    tensor=row.tensor, offset=row.offset,
                    ap=[[0, 64]] + list(row.ap),
                )
                nc.sync.dma_start(out=rb, in_=rb_src)
                a_t = apool.tile([128, L], F32R, tag=f"a{p}")
                nc.vector.tensor_tensor(
                    out=a_t[0:64, :], in0=aun[0:64, 0:1024], in1=rb[0:64, 0:1024],
                    op=ALU.mult,
                )
                nc.vector.tensor_tensor(
                    out=a_t[64:128, :], in0=aun[0:64, 1024:2048],
                    in1=rb[0:64, 1024:2048], op=ALU.mult,
                )
                ach.append(a_t)

        # ---- proj + bias + residual ----------------------------------------
        with contextlib.ExitStack() as pctx:
            pps = pctx.enter_context(tc.tile_pool(name="pr_ps", bufs=2, space="PSUM"))
            opool = pctx.enter_context(tc.tile_pool(name="o", bufs=2))
            for m in range(NCHUNK):
                pt = pps.tile([128, L], F32, tag="prps")
                for n in range(2):
                    for c in range(NCHUNK):
                        nc.tensor.matmul(
                            pt[:, n * 512:(n + 1) * 512],
                            lhsT=pw[c][:, m * 128:(m + 1) * 128],
                            rhs=ach[c][:, n * 512:(n + 1) * 512],
                            start=(c == 0), stop=(c == NCHUNK - 1),
                        )
                ot = opool.tile([128, L], F32, tag="ot")
                nc.vector.scalar_tensor_tensor(
                    out=ot, in0=pt, scalar=projb[:, m:m + 1], in1=xs[m],
                    op0=ALU.add, op1=ALU.add,
                )
                nc.sync.dma_start(
                    out=out_d.ap()[m * 128:(m + 1) * 128, :], in_=ot,
                )

    if split_waits:
        _split_excess_waits(nc)
    return nc


def prep_inputs(x, gn_w, gn_b, qkv_w, qkv_b, proj_w, proj_b):
    """Host-side prep: permute/scale QKV weights, fold biases, GN indicators."""
    x = np.ascontiguousarray(np.asarray(x, dtype=np.float32)).reshape(B, C, L)
    qkv_w = np.asarray(qkv_w, dtype=np.float32)
    qkv_b = np.asarray(qkv_b, dtype=np.float32)
    proj_w = np.asarray(proj_w, dtype=np.float32)
    proj_b = np.asarray(proj_b, dtype=np.float32)
    gn_w = np.asarray(gn_w, dtype=np.float32)
    gn_b = np.asarray(gn_b, dtype=np.float32)

    # output-row permutation: q pair-chunks, k pair-chunks, v pair-chunks
    perm = np.empty(3 * C, dtype=np.int64)
    pos = 0
    for part in range(3):             # 0=q, 1=k, 2=v
        for h in range(NH):
            rows = h * 3 * CH + part * CH + np.arange(CH)
            perm[pos:pos + CH] = rows
            pos += CH
    w_perm = qkv_w[perm, :].copy()
    b_perm = qkv_b[perm].copy()
    w_perm[0:C] *= 0.125              # fold softmax scale^2 into q
    b_perm[0:C] *= 0.125

    qkvt = np.ascontiguousarray(w_perm.T)                      # [C, 3C]
    qkb = np.ascontiguousarray(b_perm[0:2 * C].reshape(8, 128).T)  # [128, 8]
    bv = b_perm[2 * C:3 * C]                                   # v bias, head-major == channel order
    projt = np.ascontiguousarray(proj_w.T)                     # [C, C]
    projb = np.ascontiguousarray(
        (proj_b + proj_w @ bv).reshape(NCHUNK, 128).T)         # [128, 4]
    gnw_t = np.ascontiguousarray(gn_w.reshape(NCHUNK, 128).T)  # [128, 4]
    gnb_t = np.ascontiguousarray(gn_b.reshape(NCHUNK, 128).T)

    gnind = np.zeros((128, NCHUNK * NG), np.float32)
    gnexp = np.zeros((NG, NCHUNK * 128), np.float32)
    for c in range(NCHUNK):
        for p in range(128):
            g = (c * 128 + p) // GS
            gnind[p, c * NG + g] = 1.0 / GS
            gnexp[g, c * 128 + p] = 1.0
    shared = {
        "qkvt": qkvt, "qkb": qkb, "projt": projt, "projb": projb,
        "gnw": gnw_t, "gnb": gnb_t, "gnind": gnind, "gnexp": gnexp,
    }
    in_maps = [
        {"x": np.ascontiguousarray(x[i]), **shared} for i in range(N_CORES)
    ]
    return in_maps


_NC_CACHE = {}


def _get_nc():
    if "nc" not in _NC_CACHE:
        _NC_CACHE["nc"] = build_nc()
    return _NC_CACHE["nc"]


def kernel(x, gn_w, gn_b, qkv_w, qkv_b, proj_w, proj_b, _trace=False, _tmpdir=None):
    nc = _get_nc()
    in_maps = prep_inputs(x, gn_w, gn_b, qkv_w, qkv_b, proj_w, proj_b)
    res = run_bass_kernel_spmd(
        nc, in_maps, core_ids=list(range(N_CORES)), trace=_trace, tmpdir=_tmpdir,
    )
    out = np.stack([res.results[i]["out"] for i in range(N_CORES)], axis=0)
    out = out.reshape(B, C, HH, WW).astype(np.float32)
    if _trace:
        kernel.last_results = res
    return out


# revision 17
# speedup vs baseline: 1.2332x; 1.0148x over previous
"""Trainium2 Bass kernel for an AttentionBlock (GroupNorm + QKV + MHA + proj
+ residual), data-parallel over the batch across 8 NeuronCores.

Contract: kernel(**inputs) takes the FULL inputs of reference.setup_inputs()
and returns the FULL [8, 512, 32, 32] float32 output.

Per-core layout (core i handles batch element i, x viewed as [C=512, L=1024]):
  - GroupNorm(32 groups) via bn_stats per channel + tiny group-reduce matmuls.
  - QKV matmul in float32r with output rows permuted into pair-chunks
    [q0;q1]..[q6;q7], [k0;k1].., [v0;v1].. so that the two heads of a pair
    occupy partitions 0:64 / 64:128 of one 128-partition chunk.
  - scoresT[s, t] = k^T q per head, two heads packed in the PE array
    concurrently (K=64 row tiling via base partitions 0 / 64).
  - softmax without max-subtraction (logits are O(1) for this model):
    exp on the Scalar engine straight out of PSUM (fp32 -> bf16, FD=2048).
  - AV matmul with stationary operand [vT | ones] so rows 64:128 of the
    output accumulate sum(exp) per (head, t); normalization is a DVE divide.
  - proj matmul in float32r + fused (bias + residual) via scalar_tensor_tensor.
"""

import contextlib

import numpy as np

import concourse.bass as bass
import concourse.tile as tile
from concourse import mybir
from concourse.bass_utils import run_bass_kernel_spmd

F32 = mybir.dt.float32
F32R = mybir.dt.float32r
BF16 = mybir.dt.bfloat16
FT = mybir.ActivationFunctionType
ALU = mybir.AluOpType

B, C, HH, WW = 8, 512, 32, 32
L = HH * WW            # 1024
NH = 8                 # heads
CH = C // NH           # 64 channels per head
NG = 32                # groupnorm groups
GS = C // NG           # 16 channels per group
EPS = 1e-5
NCHUNK = C // 128      # 4 partition chunks of channels
N_CORES = 8


def _split_excess_waits(nc, default_max=1, ctrl_max=1):
    """walrus only encodes 1 sync wait on CTRL-like instructions (Drain/NoOp)
    and 2 on regular ones; split extra waits onto preceding NoOp carriers."""
    n_split = 0
    for f in nc.m.functions:
        for bb in f.blocks:
            insts = bb.instructions
            i = 0
            while i < len(insts):
                inst = insts[i]
                si = inst.sync_info
                cap = (
                    ctrl_max
                    if isinstance(inst, (mybir.InstDrain, mybir.InstNoOp))
                    else default_max
                )
                if si is not None and si.on_wait and len(si.on_wait) > cap:
                    waits = list(si.on_wait)
                    keep, extra = waits[-cap:], waits[:-cap]
                    carriers = [
                        mybir.InstNoOp(
                            name=f"{inst.name}-wsplit-{j}",
                            engine=inst.engine,
                            sync_info=mybir.SyncInfo(
                                on_wait=[w], on_update=[]
                            ),
                            bass_nofuse=True,
                        )
                        for j, w in enumerate(extra)
                    ]
                    inst.sync_info = mybir.SyncInfo(
                        on_wait=keep, on_update=list(si.on_update or [])
                    )
                    for k, c in enumerate(carriers):
                        insts.insert(i + k, c)
                    i += len(carriers)
                    n_split += 1
                i += 1
    return n_split


def build_nc(split_waits=True):
    nc = bass.Bass("TRN2", debug=False)

    x_d = nc.dram_tensor("x", [C, L], F32, kind="ExternalInput")
    qkvt_d = nc.dram_tensor("qkvt", [C, 3 * C], F32R, kind="ExternalInput")
    qkb_d = nc.dram_tensor("qkb", [128, 8], F32, kind="ExternalInput")
    projt_d = nc.dram_tensor("projt", [C, C], F32R, kind="ExternalInput")
    projb_d = nc.dram_tensor("projb", [128, NCHUNK], F32, kind="ExternalInput")
    gnw_d = nc.dram_tensor("gnw", [128, NCHUNK], F32, kind="ExternalInput")
    gnb_d = nc.dram_tensor("gnb", [128, NCHUNK], F32, kind="ExternalInput")
    gnind_d = nc.dram_tensor("gnind", [128, NCHUNK * NG], F32, kind="ExternalInput")
    gnexp_d = nc.dram_tensor("gnexp", [NG, NCHUNK * 128], F32, kind="ExternalInput")
    out_d = nc.dram_tensor("out", [C, L], F32, kind="ExternalOutput")
    ses_d = nc.dram_tensor("sesdram", [NCHUNK, 2, 2048], F32)

    with tile.TileContext(nc) as tc, contextlib.ExitStack() as top:
        consts = top.enter_context(tc.tile_pool(name="consts", bufs=1))
        xpool = top.enter_context(tc.tile_pool(name="x", bufs=1))
        qkpool = top.enter_context(tc.tile_pool(name="qk", bufs=1))
        vtpool = top.enter_context(tc.tile_pool(name="vt", bufs=1))
        apool = top.enter_context(tc.tile_pool(name="a", bufs=1))
        qkv_stack = contextlib.ExitStack()
        wqpool = qkv_stack.enter_context(tc.tile_pool(name="wq", bufs=1))

        # ---- input loads (x first: GroupNorm is the critical path) ----------
        xs = []
        for c in range(NCHUNK):
            t = xpool.tile([128, L], F32, tag=f"x{c}")
            nc.sync.dma_start(out=t, in_=x_d.ap()[c * 128:(c + 1) * 128, :])
            xs.append(t)
        gnw = consts.tile([128, NCHUNK], F32)
        nc.sync.dma_start(out=gnw, in_=gnw_d.ap())
        gnb = consts.tile([128, NCHUNK], F32)
        nc.sync.dma_start(out=gnb, in_=gnb_d.ap())
        gnind = consts.tile([128, NCHUNK * NG], F32)
        nc.sync.dma_start(out=gnind, in_=gnind_d.ap())
        gnexp = consts.tile([NG, NCHUNK * 128], F32)
        nc.sync.dma_start(out=gnexp, in_=gnexp_d.ap())
        epsv = consts.tile([NG, 1], F32)
        nc.vector.memset(epsv, EPS)
        # prefetch the Sqrt activation table while DMAs run
        sqrt_warm = consts.tile([NG, 1], F32)
        nc.scalar.activation(out=sqrt_warm, in_=epsv, func=FT.Sqrt)

        wq = []
        for c in range(NCHUNK):
            t = wqpool.tile([128, 3 * C], F32R, tag=f"wq{c}")
            nc.sync.dma_start(out=t, in_=qkvt_d.ap()[c * 128:(c + 1) * 128, :])
            wq.append(t)
        pw = []
        for c in range(NCHUNK):
            t = consts.tile([128, C], F32R, tag=f"pw{c}")
            nc.sync.dma_start(out=t, in_=projt_d.ap()[c * 128:(c + 1) * 128, :])
            pw.append(t)
        qkb = consts.tile([128, 8], F32)
        nc.sync.dma_start(out=qkb, in_=qkb_d.ap())
        projb = consts.tile([128, NCHUNK], F32)
        nc.sync.dma_start(out=projb, in_=projb_d.ap())

        # ---- GroupNorm ------------------------------------------------------
        with contextlib.ExitStack() as gctx:
            gsb = gctx.enter_context(tc.tile_pool(name="gn_sb", bufs=1))
            gps = gctx.enter_context(tc.tile_pool(name="gn_ps", bufs=2, space="PSUM"))

            stats3 = gsb.tile([128, NCHUNK, 3], F32)
            for c in range(NCHUNK):
                st6 = gsb.tile([128, 2, 6], F32, tag="st6")
                nc.vector.bn_stats(out=st6[:, 0, :], in_=xs[c][:, 0:512])
                nc.vector.bn_stats(out=st6[:, 1, :], in_=xs[c][:, 512:1024])
                mv = gsb.tile([128, 2], F32, tag="mv")
                nc.vector.bn_aggr(out=mv, in_=st6)
                nc.vector.tensor_copy(stats3[:, c, 0:2], mv)
                nc.vector.tensor_tensor(
                    out=stats3[:, c, 2:3], in0=mv[:, 0:1], in1=mv[:, 0:1],
                    op=ALU.mult,
                )
            gst = gps.tile([NG, 3], F32)
            for c in range(NCHUNK):
                nc.tensor.matmul(
                    gst,
                    lhsT=gnind[:, c * NG:(c + 1) * NG],
                    rhs=stats3[:, c, :],
                    start=(c == 0), stop=(c == NCHUNK - 1),
                )
            # group stats: [gmean, mean_of_var, mean_of_mean2]
            grs = gsb.tile([NG, 3], F32)
            nc.vector.tensor_copy(grs, gst)
            gvar = gsb.tile([NG, 1], F32)
            nc.vector.tensor_tensor(out=gvar, in0=grs[:, 1:2], in1=grs[:, 2:3], op=ALU.add)
            m2 = gsb.tile([NG, 1], F32)
            nc.vector.tensor_tensor(out=m2, in0=grs[:, 0:1], in1=grs[:, 0:1], op=ALU.mult)
            nc.vector.tensor_tensor(out=gvar, in0=gvar, in1=m2, op=ALU.subtract)
            # grs2: col0 = gmean, col1 = rstd
            grs2 = gsb.tile([NG, 2], F32)
            nc.vector.tensor_copy(grs2[:, 0:1], grs[:, 0:1])
            sd = gsb.tile([NG, 1], F32)
            nc.scalar.activation(out=sd, in_=gvar, func=FT.Sqrt, bias=epsv, scale=1.0)
            nc.vector.reciprocal(out=grs2[:, 1:2], in_=sd)

            alpha = gsb.tile([128, NCHUNK], F32)
            beta = gsb.tile([128, NCHUNK], F32)
            for c in range(NCHUNK):
                cs = gps.tile([128, 2], F32, tag="cs")
                nc.tensor.matmul(
                    cs,
                    lhsT=gnexp[:, c * 128:(c + 1) * 128],
                    rhs=grs2,
                    start=True, stop=True,
                )
                nc.vector.tensor_tensor(
                    out=alpha[:, c:c + 1], in0=cs[:, 1:2], in1=gnw[:, c:c + 1],
                    op=ALU.mult,
                )
                ngm = gsb.tile([128, 1], F32, tag="ngm")
                nc.vector.tensor_scalar(
                    out=ngm, in0=cs[:, 0:1], scalar1=-1.0, scalar2=None, op0=ALU.mult,
                )
                nc.vector.scalar_tensor_tensor(
                    out=beta[:, c:c + 1], in0=ngm, scalar=alpha[:, c:c + 1],
                    in1=gnb[:, c:c + 1], op0=ALU.mult, op1=ALU.add,
                )
            # xn = x * alpha + beta  (float32r so it can feed f32r matmuls)
            xn = []
            for c in range(NCHUNK):
                t = wqpool.tile([128, L], F32R, tag=f"xn{c}")
                nc.vector.tensor_scalar(
                    out=t, in0=xs[c], scalar1=alpha[:, c:c + 1],
                    scalar2=beta[:, c:c + 1], op0=ALU.mult, op1=ALU.add,
                )
                xn.append(t)
            # prefetch the Exp activation table (overlaps the QKV matmuls)
            exp_warm = gsb.tile([NG, 1], F32)
            nc.scalar.activation(out=exp_warm, in_=sd, func=FT.Exp)

        # ---- QKV (f32r) interleaved per head-pair ---------------------------
        # q/k: standard [o-chunk, l] outputs, drained with bias -> bf16.
        # v: computed TRANSPOSED (xn stationary, v-weight cols moving) so the
        #    AV stationary operand [vT | ones] needs no separate transpose.
        qp, kp = [None] * 4, [None] * 4
        vth = []
        for h in range(NH):
            t = vtpool.tile([128, 8, 128], BF16, tag=f"vt{h}")
            nc.gpsimd.memset(t[:, :, 64:128], 1.0)
            vth.append(t)
        qps = qkv_stack.enter_context(tc.tile_pool(name="qkv_ps", bufs=3, space="PSUM"))
        vps = qkv_stack.enter_context(tc.tile_pool(name="vt_ps", bufs=2, space="PSUM"))
        for p in range(NCHUNK):
            for m in (p, 4 + p):
                pt = qps.tile([128, L], F32, tag="qkvps")
                for n in range(2):
                    for c in range(NCHUNK):
                        nc.tensor.matmul(
                            pt[:, n * 512:(n + 1) * 512],
                            lhsT=wq[c][:, m * 128:(m + 1) * 128],
                            rhs=xn[c][:, n * 512:(n + 1) * 512],
                            start=(c == 0), stop=(c == NCHUNK - 1),
                        )
                t = qkpool.tile([128, L], BF16, tag=f"qk{m}")
                nc.vector.tensor_scalar(
                    out=t, in0=pt, scalar1=qkb[:, m:m + 1], scalar2=None,
                    op0=ALU.add,
                )
                (qp if m < 4 else kp)[m % 4] = t
            # vT for this pair: out block [s-block, c-pair] per 4-block group
            vcols = slice(1024 + p * 128, 1024 + (p + 1) * 128)
            for g in range(2):
                vt_ps = vps.tile([128, 4, 128], F32, tag="vtps")
                for b in range(4):
                    i = g * 4 + b
                    for c in range(NCHUNK):
                        nc.tensor.matmul(
                            vt_ps[:, b, :],
                            lhsT=xn[c][:, i * 128:(i + 1) * 128],
                            rhs=wq[c][:, vcols],
                            start=(c == 0), stop=(c == NCHUNK - 1),
                        )
                nc.vector.tensor_copy(
                    vth[2 * p][:, g * 4:(g + 1) * 4, 0:64], vt_ps[:, :, 0:64],
                )
                nc.vector.tensor_copy(
                    vth[2 * p + 1][:, g * 4:(g + 1) * 4, 0:64], vt_ps[:, :, 64:128],
                )
        qkv_stack.close()

        # ---- attention per head-pair ---------------------------------------
        ach = []
        with contextlib.ExitStack() as actx:
            sps = actx.enter_context(tc.tile_pool(name="sc_ps", bufs=2, space="PSUM"))
            wtp_pool = actx.enter_context(tc.tile_pool(name="wt", bufs=2))
            aupool = actx.enter_context(tc.tile_pool(name="aun", bufs=2))
            rpool = actx.enter_context(tc.tile_pool(name="rcp", bufs=2))
            for p in range(NCHUNK):
                wt = wtp_pool.tile([128, 8, 2048], BF16, tag="wt")
                for i in range(8):
                    st = sps.tile([128, 2048], F32, tag="scav")
                    for n in range(2):
                        for hb, off in ((0, 0), (64, 1024)):
                            nc.tensor.matmul(
                                st[:, off + n * 512: off + (n + 1) * 512],
                                lhsT=kp[p][hb:hb + 64, i * 128:(i + 1) * 128],
                                rhs=qp[p][hb:hb + 64, n * 512:(n + 1) * 512],
                                start=True, stop=True,
                                tile_position=(hb, 0),
                            )
                    nc.scalar.activation(out=wt[:, i, :], in_=st, func=FT.Exp)
                av = sps.tile([128, 2048], F32, tag="scav")
                for i in range(8):
                    for hi, off in ((2 * p, 0), (2 * p + 1, 1024)):
                        for n in range(2):
                            nc.tensor.matmul(
                                av[:, off + n * 512: off + (n + 1) * 512],
                                lhsT=vth[hi][:, i, :],
                                rhs=wt[:, i, off + n * 512: off + (n + 1) * 512],
                                start=(i == 0), stop=(i == 7),
                            )
                # [a_un; sumexp] out of PSUM in one copy, then spread-reciprocal
                aun = aupool.tile([65, 2048], F32, tag="aun")
                nc.vector.tensor_copy(aun, av[0:65, :])
                nc.sync.dma_start(out=ses_d.ap()[p, 0, :], in_=aun[64:65, :])
                sesw = rpool.tile([128, 16], F32, tag="sesw")
                nc.sync.dma_start(
                    out=sesw,
                    in_=ses_d.ap()[p, 0, :].rearrange("(p f) -> p f", p=128),
                )
                nc.vector.reciprocal(out=sesw, in_=sesw)
                nc.sync.dma_start(out=ses_d.ap()[p, 1, :], in_=sesw)
                rb = rpool.tile([64, 2048], F32, tag="rb")
                row = ses_d.ap()[p, 1, :]
                rb_src = bass.AP(
                    tensor=row.tensor, offset=row.offset,
                    ap=[[0, 64]] + list(row.ap),
                )
                nc.sync.dma_start(out=rb, in_=rb_src)
                a_t = apool.tile([128, L], F32R, tag=f"a{p}")
                nc.vector.tensor_tensor(
                    out=a_t[0:64, :], in0=aun[0:64, 0:1024], in1=rb[0:64, 0:1024],
                    op=ALU.mult,
                )
                nc.vector.tensor_tensor(
                    out=a_t[64:128, :], in0=aun[0:64, 1024:2048],
                    in1=rb[0:64, 1024:2048], op=ALU.mult,
                )
                ach.append(a_t)

        # ---- proj + bias + residual ----------------------------------------
        with contextlib.ExitStack() as pctx:
            pps = pctx.enter_context(tc.tile_pool(name="pr_ps", bufs=2, space="PSUM"))
            opool = pctx.enter_context(tc.tile_pool(name="o", bufs=2))
            for m in range(NCHUNK):
                pt = pps.tile([128, L], F32, tag="prps")
                for n in range(2):
                    for c in range(NCHUNK):
                        nc.tensor.matmul(
                            pt[:, n * 512:(n + 1) * 512],
                            lhsT=pw[c][:, m * 128:(m + 1) * 128],
                            rhs=ach[c][:, n * 512:(n + 1) * 512],
                            start=(c == 0), stop=(c == NCHUNK - 1),
                        )
                ot = opool.tile([128, L], F32, tag="ot")
                nc.vector.scalar_tensor_tensor(
                    out=ot, in0=pt, scalar=projb[:, m:m + 1], in1=xs[m],
                    op0=ALU.add, op1=ALU.add,
                )
                nc.sync.dma_start(
                    out=out_d.ap()[m * 128:(m + 1) * 128, :], in_=ot,
                )

    if split_waits:
        _split_excess_waits(nc)
    return nc


def prep_inputs(x, gn_w, gn_b, qkv_w, qkv_b, proj_w, proj_b):
    """Host-side prep: permute/scale QKV weights, fold biases, GN indicators."""
    x = np.ascontiguousarray(np.asarray(x, dtype=np.float32)).reshape(B, C, L)
    qkv_w = np.asarray(qkv_w, dtype=np.float32)
    qkv_b = np.asarray(qkv_b, dtype=np.float32)
    proj_w = np.asarray(proj_w, dtype=np.float32)
    proj_b = np.asarray(proj_b, dtype=np.float32)
    gn_w = np.asarray(gn_w, dtype=np.float32)
    gn_b = np.asarray(gn_b, dtype=np.float32)

    # output-row permutation: q pair-chunks, k pair-chunks, v pair-chunks
    perm = np.empty(3 * C, dtype=np.int64)
    pos = 0
    for part in range(3):             # 0=q, 1=k, 2=v
        for h in range(NH):
            rows = h * 3 * CH + part * CH + np.arange(CH)
            perm[pos:pos + CH] = rows
            pos += CH
    w_perm = qkv_w[perm, :].copy()
    b_perm = qkv_b[perm].copy()
    w_perm[0:C] *= 0.125              # fold softmax scale^2 into q
    b_perm[0:C] *= 0.125

    qkvt = np.ascontiguousarray(w_perm.T)                      # [C, 3C]
    qkb = np.ascontiguousarray(b_perm[0:2 * C].reshape(8, 128).T)  # [128, 8]
    bv = b_perm[2 * C:3 * C]                                   # v bias, head-major == channel order
    projt = np.ascontiguousarray(proj_w.T)                     # [C, C]
    projb = np.ascontiguousarray(
        (proj_b + proj_w @ bv).reshape(NCHUNK, 128).T)         # [128, 4]
    gnw_t = np.ascontiguousarray(gn_w.reshape(NCHUNK, 128).T)  # [128, 4]
    gnb_t = np.ascontiguousarray(gn_b.reshape(NCHUNK, 128).T)

    gnind = np.zeros((128, NCHUNK * NG), np.float32)
    gnexp = np.zeros((NG, NCHUNK * 128), np.float32)
    for c in range(NCHUNK):
        for p in range(128):
            g = (c * 128 + p) // GS
            gnind[p, c * NG + g] = 1.0 / GS
            gnexp[g, c * 128 + p] = 1.0
    shared = {
        "qkvt": qkvt, "qkb": qkb, "projt": projt, "projb": projb,
        "gnw": gnw_t, "gnb": gnb_t, "gnind": gnind, "gnexp": gnexp,
    }
    in_maps = [
        {"x": np.ascontiguousarray(x[i]), **shared} for i in range(N_CORES)
    ]
    return in_maps


_NC_CACHE = {}


def _get_nc():
    if "nc" not in _NC_CACHE:
        _NC_CACHE["nc"] = build_nc()
    return _NC_CACHE["nc"]


def kernel(x, gn_w, gn_b, qkv_w, qkv_b, proj_w, proj_b, _trace=False, _tmpdir=None):
    nc = _get_nc()
    in_maps = prep_inputs(x, gn_w, gn_b, qkv_w, qkv_b, proj_w, proj_b)
    res = run_bass_kernel_spmd(
        nc, in_maps, core_ids=list(range(N_CORES)), trace=_trace, tmpdir=_tmpdir,
    )
    out = np.stack([res.results[i]["out"] for i in range(N_CORES)], axis=0)
    out = out.reshape(B, C, HH, WW).astype(np.float32)
    if _trace:
        kernel.last_results = res
    return out
